# revision 17
# baseline (speedup 1.0000x reference)
"""Trainium2 Bass kernel for DifferentiableBoxParser.

Per (b, k): softmax over the 256x256 score map (T=0.1) -> expected coords
(y, x); soft-ceil + smooth-clamp; int cast; gather offsets at the resulting
index; pts = (coords + offset) * 4.

Device does the heavy part (streaming the 128 MiB score_map and computing,
per map, the softmax partial sums Z, Sy-parts, Sx). Host finishes the tiny
per-pair scalar math and the 2-element-per-pair offset gather (reading the
256 MiB offset_map on device would be pure waste: only 1024 of its elements
are needed).

Sharding: data-parallel over batch, 8 batches per core (64 maps per core).

Device layout per core: score reshaped to [1024, 4096]; group g in [0,8)
covers 8 maps; SBUF tile [128, 4096] with partition p = 16*j + s (j = map in
group, s = h-high), free f = h_low*256 + w with h = 16*s + h_low. Per 512-col
chunk q (h_low = 2q + b, b = (f%512)//256), a matmul with block-diagonal
weights accumulates into PSUM [16, 512]:
  row 2j   : colsum_j[f']  = sum_s E
  row 2j+1 : sum_s (16s + 2q) E
Finalize per group on DVE: Z = sum(row 2j); B = sum(row 2j, f' in [256,512));
S16 = sum(row 2j+1); Xw = sum(f'%256 * row 2j).
Then y = (S16 + B)/Z, x = Xw/Z on host. exp computed as exp(10*x - 40)
(softmax is shift-invariant; keeps f32 range safe).

exp output and matmul weights are bfloat16: the PE streams bf16 at 2.4 GHz
vs 1.2 GHz for f32/f32r, halving TensorE time so the whole compute pipeline
hides under the DMA stream (the kernel is HBM-bandwidth-bound). All weight
values (16s + h_low <= 255) are integers exactly representable in bf16;
PSUM accumulation stays f32. bf16 quantization of the exp values (~2^-9
relative) perturbs the expected coords by well under REFINE_DELTA; pairs
whose clamped coords land within REFINE_DELTA of a half-integer rounding
boundary are recomputed exactly on host in float64 so the non-differentiable
int cast can't flip.
"""
import sys
import numpy as np

for _p in ("/opt/trn_rl_repo", "/opt/pypackages"):
    if _p not in sys.path:
        sys.path.append(_p)

import concourse.bacc as bacc
import concourse.tile as tile
from concourse import mybir
from concourse.bass_utils import run_bass_kernel_spmd

N_CORES = 8
BS, K, HO, WO = 64, 8, 256, 256
STRIDE = 4
TEMPERATURE = 0.1
SHARPNESS = 10.0
SMOOTHNESS = 0.1
EXP_SHIFT = -42.0

NPG = 8            # maps per group
NGROUP = 8         # groups per core (8 maps/group * 8 groups = 64 maps/core)
P = 128
FD = 4096
NCHUNK = 8
MM_DT = mybir.dt.float16
REFINE_DELTA = 0.05

_CACHE = {}


def _build_nc():
    nc = bacc.Bacc(None, target_bir_lowering=False, debug=False)
    score = nc.dram_tensor("score", [NGROUP * P, FD], mybir.dt.float32,
                           kind="ExternalInput")
    wmat = nc.dram_tensor("wmat", [P, NCHUNK, 16], MM_DT, kind="ExternalInput")
    wvin = nc.dram_tensor("wvin", [16, 2, 256], mybir.dt.float32, kind="ExternalInput")
    stats = nc.dram_tensor("stats", [16, NGROUP, 3], mybir.dt.float32,
                           kind="ExternalOutput")

    with tile.TileContext(nc) as tc:
        with (
            tc.tile_pool(name="singles", bufs=1) as singles,
            tc.tile_pool(name="xin", bufs=4) as xin,
            tc.tile_pool(name="expo", bufs=3) as expo,
            tc.tile_pool(name="fin", bufs=3) as fin,
            tc.tile_pool(name="psum", bufs=4, space="PSUM") as psum_pool,
        ):
            xts = []
            for g in range(2):
                xt = xin.tile([P, FD], mybir.dt.float32)
                # first two groups ride the gpsimd SWDGE: its engine program
                # clears boilerplate ~2us before the sync ring's, so the
                # stream starts earlier
                nc.gpsimd.dma_start(out=xt[:], in_=score[g * P:(g + 1) * P, :])
                xts.append(xt)

            wt = singles.tile([P, NCHUNK, 16], MM_DT)
            nc.gpsimd.dma_start(out=wt[:], in_=wmat[:])
            bias_t = singles.tile([P, 1], mybir.dt.float32)
            nc.vector.memset(bias_t[:], EXP_SHIFT)
            wvec = singles.tile([16, 2, 256], mybir.dt.float32)
            nc.gpsimd.dma_start(out=wvec[:], in_=wvin[:])

            for g in range(NGROUP):
                last = g == NGROUP - 1
                # DMA in >=1 MiB chunks (max-bandwidth threshold); the last
                # group tapers so the final exp+matmul covers only 512 cols
                # after the last byte lands
                if last:
                    dsplits = [0, 2048, 3072, 3584, 4096]
                    esplits = [0, 2048, 3072, 3584, 4096]
                else:
                    dsplits = [0, 2048, 4096]
                    esplits = [0, 2048, 4096]
                if g < 2:
                    xt = xts[g]
                else:
                    xt = xin.tile([P, FD], mybir.dt.float32)
                    for d0, d1 in zip(dsplits[:-1], dsplits[1:]):
                        nc.sync.dma_start(out=xt[:, d0:d1],
                                          in_=score[g * P:(g + 1) * P, d0:d1])
                ps = psum_pool.tile([16, 512], mybir.dt.float32)
                cw = 512
                # one et tile PER exp chunk: a shared per-group tile makes
                # the tile tracker serialize exp(chunk k+1) behind the
                # matmuls still reading chunk k (whole-tile WAR), killing
                # the ACT/PE overlap
                for ci, (a0, a1) in enumerate(zip(esplits[:-1], esplits[1:])):
                    tg = f"lt{ci}" if last else f"et{ci}"
                    et = expo.tile([P, a1 - a0], MM_DT, tag=tg)
                    nc.scalar.activation(out=et[:],
                                         in_=xt[:, a0:a1],
                                         func=mybir.ActivationFunctionType.Exp,
                                         bias=bias_t[:], scale=1.0 / TEMPERATURE)
                    for q in range(a0 // cw, a1 // cw):
                        nc.tensor.matmul(
                            ps[:, 0:cw],
                            wt[:, q, :],
                            et[:, q * cw - a0:(q + 1) * cw - a0],
                            start=(q == 0), stop=(q == NCHUNK - 1),
                        )

                tmp = fin.tile([16, 512], mybir.dt.float32)
                if last:
                    # final drain: DVE does Xw (mul+reduce) while ACT —
                    # idle after its last exp — computes Z/S16 and B via
                    # Copy+accum_out on separate tiles; three independent
                    # stats DMAs go out as each stat completes
                    obx = fin.tile([16, 1], mybir.dt.float32, tag="obx")
                    obz = fin.tile([16, 1], mybir.dt.float32, tag="obz")
                    obb = fin.tile([16, 1], mybir.dt.float32, tag="obb")
                    nc.vector.tensor_mul(tmp[:], ps[:],
                                         wvec.rearrange("p a b -> p (a b)")[:])
                    nc.vector.reduce_sum(obx[:], tmp[:],
                                         axis=mybir.AxisListType.X)
                    junk = fin.tile([16, 512], mybir.dt.float32, tag="junk")
                    nc.scalar.activation(out=junk[:], in_=ps[:],
                                         func=mybir.ActivationFunctionType.Copy,
                                         accum_out=obz[:])
                    nc.scalar.activation(out=junk[:, 0:256], in_=ps[:, 256:512],
                                         func=mybir.ActivationFunctionType.Copy,
                                         accum_out=obb[:])
                    nc.sync.dma_start(out=stats[:, g, 0:1], in_=obz[:])
                    nc.sync.dma_start(out=stats[:, g, 1:2], in_=obb[:])
                    # Xw's DMA rides the gpsimd SWDGE so it doesn't queue
                    # behind the two sync-ring descriptor gens
                    nc.gpsimd.dma_start(out=stats[:, g, 2:3], in_=obx[:])
                else:
                    ob = fin.tile([16, 3], mybir.dt.float32, tag="ob")
                    nc.vector.reduce_sum(ob[:, 0:1], ps[:, 0:cw],
                                         axis=mybir.AxisListType.X)
                    nc.vector.reduce_sum(ob[:, 1:2], ps[:, 256:512],
                                         axis=mybir.AxisListType.X)
                    nc.vector.tensor_mul(tmp[:, 0:cw], ps[:, 0:cw],
                                         wvec.rearrange("p a b -> p (a b)")[:, 0:cw])
                    nc.vector.reduce_sum(ob[:, 2:3], tmp[:, 0:cw],
                                         axis=mybir.AxisListType.X)
                    # per-group stats write on the otherwise-idle gpsimd
                    # SWDGE so it never head-of-line blocks the sync ring
                    # streaming score
                    nc.gpsimd.dma_start(out=stats[:, g, :], in_=ob[:])

    nc.compile()
    return nc


def _weights():
    W = np.zeros((P, NCHUNK, 16), dtype=np.float32)
    s = np.arange(16)
    for j in range(NPG):
        W[16 * j + s, :, 2 * j] = 1.0
        for q in range(NCHUNK):          # 512-wide chunks: h_low = 2q + b
            W[16 * j + s, q, 2 * j + 1] = (16 * s + 2 * q).astype(np.float32)
    return W.astype(np.float16)


def _get_compiled():
    if "nc" not in _CACHE:
        _CACHE["nc"] = _build_nc()
        _CACHE["W"] = _weights()
        wv = np.tile(np.arange(256, dtype=np.float32)[None, None, :], (16, 2, 1))
        _CACHE["WV"] = wv
    return _CACHE["nc"], _CACHE["W"]


def _device_coords(score_map, trace=False):
    """Run the Bass kernel on 8 cores; return y, x arrays of shape [BS, K]
    (plus the BassKernelResults of the run)."""
    nc, W = _get_compiled()
    flat = np.ascontiguousarray(score_map.reshape(BS * K, 16, FD))
    bpc = BS // N_CORES                      # batches per core
    mpc = bpc * K                            # maps per core
    in_maps = []
    for c in range(N_CORES):
        shard = flat[c * mpc:(c + 1) * mpc].reshape(NGROUP * P, FD)
        in_maps.append({"score": shard, "wmat": W, "wvin": _CACHE["WV"]})
    res = run_bass_kernel_spmd(nc, in_maps, list(range(N_CORES)), trace=trace)

    ys = np.empty((BS, K), dtype=np.float64)
    xs = np.empty((BS, K), dtype=np.float64)
    for c in range(N_CORES):
        st = res.results[c]["stats"].astype(np.float64)   # [16, 8, 3]
        rows = np.arange(NPG)
        Z = st[2 * rows, :, 0]               # [j, g]
        S16 = st[2 * rows + 1, :, 0]
        B = st[2 * rows, :, 1]
        Xw = st[2 * rows, :, 2]
        y = (S16 + B) / Z                    # [j, g]
        x = Xw / Z
        # map (g, j) -> core-local pair index 8g + j -> (b_local, k)
        y = y.T.reshape(mpc)                 # [g, j] -> pair-major
        x = x.T.reshape(mpc)
        ys[c * bpc:(c + 1) * bpc] = y.reshape(bpc, K)
        xs[c * bpc:(c + 1) * bpc] = x.reshape(bpc, K)
    return ys, xs, res


def _exact_coords(sm64):
    """Float64 softmax expected coords for one [HO, WO] score map."""
    z = sm64 / TEMPERATURE
    z = z - z.max()
    e = np.exp(z)
    Z = e.sum()
    y = (e.sum(axis=1) * np.arange(HO)).sum() / Z
    x = (e.sum(axis=0) * np.arange(WO)).sum() / Z
    return y, x


def _soft_ceil(x):
    return x + (1.0 - 1.0 / (1.0 + np.exp(-SHARPNESS * (x - np.floor(x)))))


def _smooth_clamp(x, min_val, max_val):
    x = np.where(x < min_val,
                 min_val + SMOOTHNESS * np.tanh((x - min_val) / SMOOTHNESS), x)
    x = np.where(x > max_val,
                 max_val - SMOOTHNESS * np.tanh((max_val - x) / SMOOTHNESS), x)
    return x


def kernel(score_map, offset_map, _trace=False, _res_out=None):
    score_map = np.asarray(score_map)
    offset_map = np.asarray(offset_map)

    ys, xs, res = _device_coords(score_map, trace=_trace)
    if _res_out is not None:
        _res_out.append(res)

    coords = np.stack([ys, xs], axis=-1)          # [BS, K, 2] float64

    cc = _soft_ceil(coords)
    y_cl = _smooth_clamp(cc[..., 0], 0.0, float(HO - 1))
    x_cl = _smooth_clamp(cc[..., 1], 0.0, float(WO - 1))

    # The harness executes the reference on the same neuron jax backend,
    # where .astype(int32) rounds half-to-even (np.rint) rather than
    # truncating. Refine pairs whose clamped coords sit near a rounding
    # boundary (half-integers): the cast there is sensitive to the
    # device's bf16/f32r noise.
    fy = np.abs(y_cl - np.floor(y_cl) - 0.5)
    fx = np.abs(x_cl - np.floor(x_cl) - 0.5)
    sus = (fy < REFINE_DELTA) | (fx < REFINE_DELTA)
    for b, k in zip(*np.nonzero(sus)):
        yy, xx = _exact_coords(score_map[b, k].astype(np.float64))
        coords[b, k, 0] = yy
        coords[b, k, 1] = xx
        cc = _soft_ceil(coords[b, k])
        y_cl[b, k] = _smooth_clamp(cc[0], 0.0, float(HO - 1))
        x_cl[b, k] = _smooth_clamp(cc[1], 0.0, float(WO - 1))

    y_idx = np.rint(y_cl).astype(np.int32)
    x_idx = np.rint(x_cl).astype(np.int32)
    b_idx = np.arange(BS)[:, None]
    k_idx = np.arange(K)[None, :]
    off_y = offset_map[b_idx, 2 * k_idx, y_idx, x_idx]
    off_x = offset_map[b_idx, 2 * k_idx + 1, y_idx, x_idx]
    offset = np.stack([off_y, off_x], axis=-1)

    pts = (coords.astype(np.float32) + offset) * STRIDE
    return pts.astype(np.float32)


# revision 18
# speedup vs baseline: 1.1297x; 1.1297x over previous
"""Trainium2 Bass kernel for DifferentiableBoxParser.

Per (b, k): softmax over the 256x256 score map (T=0.1) -> expected coords
(y, x); soft-ceil + smooth-clamp; int cast; gather offsets at the resulting
index; pts = (coords + offset) * 4.

Device does the heavy part (streaming the 128 MiB score_map and computing,
per map, the softmax partial sums Z, Sy-parts, Sx). Host finishes the tiny
per-pair scalar math and the 2-element-per-pair offset gather (reading the
256 MiB offset_map on device would be pure waste: only 1024 of its elements
are needed).

Sharding: data-parallel over batch, 8 batches per core (64 maps per core).

Device layout per core: score reshaped to [1024, 4096]; group g in [0,8)
covers 8 maps; SBUF tile [128, 4096] with partition p = 16*j + s (j = map in
group, s = h-high), free f = h_low*256 + w with h = 16*s + h_low. Per 512-col
chunk q (h_low = 2q + b, b = (f%512)//256), a matmul with block-diagonal
weights accumulates into PSUM [16, 512]:
  row 2j   : colsum_j[f']  = sum_s E
  row 2j+1 : sum_s (16s + 2q) E
Finalize per group on DVE: Z = sum(row 2j); B = sum(row 2j, f' in [256,512));
S16 = sum(row 2j+1); Xw = sum(f'%256 * row 2j).
Then y = (S16 + B)/Z, x = Xw/Z on host. exp computed as exp(10*x - 40)
(softmax is shift-invariant; keeps f32 range safe).

exp output and matmul weights are bfloat16: the PE streams bf16 at 2.4 GHz
vs 1.2 GHz for f32/f32r, halving TensorE time so the whole compute pipeline
hides under the DMA stream (the kernel is HBM-bandwidth-bound). All weight
values (16s + h_low <= 255) are integers exactly representable in bf16;
PSUM accumulation stays f32. bf16 quantization of the exp values (~2^-9
relative) perturbs the expected coords by well under REFINE_DELTA; pairs
whose clamped coords land within REFINE_DELTA of a half-integer rounding
boundary are recomputed exactly on host in float64 so the non-differentiable
int cast can't flip.
"""
import sys
import numpy as np

for _p in ("/opt/trn_rl_repo", "/opt/pypackages"):
    if _p not in sys.path:
        sys.path.append(_p)

import concourse.bacc as bacc
import concourse.tile as tile
from concourse import mybir
from concourse.bass_utils import run_bass_kernel_spmd

N_CORES = 8
BS, K, HO, WO = 64, 8, 256, 256
STRIDE = 4
TEMPERATURE = 0.1
SHARPNESS = 10.0
SMOOTHNESS = 0.1
EXP_SHIFT = -42.0

NPG = 8            # maps per group
NGROUP = 8         # groups per core (8 maps/group * 8 groups = 64 maps/core)
P = 128
FD = 4096
NCHUNK = 8
MM_DT = mybir.dt.float16
REFINE_DELTA = 0.05

_CACHE = {}


def _build_nc():
    nc = bacc.Bacc(None, target_bir_lowering=False, debug=False)
    score = nc.dram_tensor("score", [NGROUP * P, FD], mybir.dt.float32,
                           kind="ExternalInput")
    wmat = nc.dram_tensor("wmat", [P, NCHUNK, 16], MM_DT, kind="ExternalInput")
    wvin = nc.dram_tensor("wvin", [16, 2, 256], mybir.dt.float32, kind="ExternalInput")
    stats = nc.dram_tensor("stats", [16, NGROUP, 3], mybir.dt.float32,
                           kind="ExternalOutput")

    with tile.TileContext(nc) as tc:
        with (
            tc.tile_pool(name="singles", bufs=1) as singles,
            tc.tile_pool(name="xin", bufs=4) as xin,
            tc.tile_pool(name="expo", bufs=3) as expo,
            tc.tile_pool(name="fin", bufs=3) as fin,
            tc.tile_pool(name="psum", bufs=4, space="PSUM") as psum_pool,
        ):
            xts = []
            for g in range(2):
                xt = xin.tile([P, FD], mybir.dt.float32)
                nc.sync.dma_start(out=xt[:], in_=score[g * P:(g + 1) * P, :])
                xts.append(xt)

            wt = singles.tile([P, NCHUNK, 16], MM_DT)
            nc.gpsimd.dma_start(out=wt[:], in_=wmat[:])
            bias_t = singles.tile([P, 1], mybir.dt.float32)
            nc.vector.memset(bias_t[:], EXP_SHIFT)
            wvec = singles.tile([16, 2, 256], mybir.dt.float32)
            nc.gpsimd.dma_start(out=wvec[:], in_=wvin[:])

            for g in range(NGROUP):
                last = g == NGROUP - 1
                # DMA in >=1 MiB chunks (max-bandwidth threshold); the last
                # group tapers so the final exp+matmul covers only 512 cols
                # after the last byte lands
                if last:
                    dsplits = [0, 2048, 3072, 3584, 4096]
                    esplits = [0, 2048, 3072, 3584, 4096]
                else:
                    dsplits = [0, 2048, 4096]
                    esplits = [0, 2048, 4096]
                if g < 2:
                    xt = xts[g]
                else:
                    xt = xin.tile([P, FD], mybir.dt.float32)
                    for d0, d1 in zip(dsplits[:-1], dsplits[1:]):
                        nc.sync.dma_start(out=xt[:, d0:d1],
                                          in_=score[g * P:(g + 1) * P, d0:d1])
                ps = psum_pool.tile([16, 512], mybir.dt.float32)
                cw = 512
                # one et tile PER exp chunk: a shared per-group tile makes
                # the tile tracker serialize exp(chunk k+1) behind the
                # matmuls still reading chunk k (whole-tile WAR), killing
                # the ACT/PE overlap
                for ci, (a0, a1) in enumerate(zip(esplits[:-1], esplits[1:])):
                    tg = f"lt{ci}" if last else f"et{ci}"
                    et = expo.tile([P, a1 - a0], MM_DT, tag=tg)
                    nc.scalar.activation(out=et[:],
                                         in_=xt[:, a0:a1],
                                         func=mybir.ActivationFunctionType.Exp,
                                         bias=bias_t[:], scale=1.0 / TEMPERATURE)
                    for q in range(a0 // cw, a1 // cw):
                        nc.tensor.matmul(
                            ps[:, 0:cw],
                            wt[:, q, :],
                            et[:, q * cw - a0:(q + 1) * cw - a0],
                            start=(q == 0), stop=(q == NCHUNK - 1),
                        )

                tmp = fin.tile([16, 512], mybir.dt.float32)
                if last:
                    # final drain: DVE does Xw (mul+reduce) while ACT —
                    # idle after its last exp — computes Z/S16 and B via
                    # Copy+accum_out on separate tiles; three independent
                    # stats DMAs go out as each stat completes
                    obx = fin.tile([16, 1], mybir.dt.float32, tag="obx")
                    obz = fin.tile([16, 1], mybir.dt.float32, tag="obz")
                    obb = fin.tile([16, 1], mybir.dt.float32, tag="obb")
                    nc.vector.tensor_mul(tmp[:], ps[:],
                                         wvec.rearrange("p a b -> p (a b)")[:])
                    nc.vector.reduce_sum(obx[:], tmp[:],
                                         axis=mybir.AxisListType.X)
                    junk = fin.tile([16, 512], mybir.dt.float32, tag="junk")
                    nc.scalar.activation(out=junk[:], in_=ps[:],
                                         func=mybir.ActivationFunctionType.Copy,
                                         accum_out=obz[:])
                    nc.scalar.activation(out=junk[:, 0:256], in_=ps[:, 256:512],
                                         func=mybir.ActivationFunctionType.Copy,
                                         accum_out=obb[:])
                    nc.sync.dma_start(out=stats[:, g, 0:1], in_=obz[:])
                    nc.sync.dma_start(out=stats[:, g, 1:2], in_=obb[:])
                    # Xw's DMA rides the gpsimd SWDGE so it doesn't queue
                    # behind the two sync-ring descriptor gens
                    nc.gpsimd.dma_start(out=stats[:, g, 2:3], in_=obx[:])
                else:
                    ob = fin.tile([16, 3], mybir.dt.float32, tag="ob")
                    nc.vector.reduce_sum(ob[:, 0:1], ps[:, 0:cw],
                                         axis=mybir.AxisListType.X)
                    nc.vector.reduce_sum(ob[:, 1:2], ps[:, 256:512],
                                         axis=mybir.AxisListType.X)
                    nc.vector.tensor_mul(tmp[:, 0:cw], ps[:, 0:cw],
                                         wvec.rearrange("p a b -> p (a b)")[:, 0:cw])
                    nc.vector.reduce_sum(ob[:, 2:3], tmp[:, 0:cw],
                                         axis=mybir.AxisListType.X)
                    # per-group stats write on the otherwise-idle gpsimd
                    # SWDGE so it never head-of-line blocks the sync ring
                    # streaming score
                    nc.gpsimd.dma_start(out=stats[:, g, :], in_=ob[:])

    nc.compile()
    return nc


def _weights():
    W = np.zeros((P, NCHUNK, 16), dtype=np.float32)
    s = np.arange(16)
    for j in range(NPG):
        W[16 * j + s, :, 2 * j] = 1.0
        for q in range(NCHUNK):          # 512-wide chunks: h_low = 2q + b
            W[16 * j + s, q, 2 * j + 1] = (16 * s + 2 * q).astype(np.float32)
    return W.astype(np.float16)


def _get_compiled():
    if "nc" not in _CACHE:
        _CACHE["nc"] = _build_nc()
        _CACHE["W"] = _weights()
        wv = np.tile(np.arange(256, dtype=np.float32)[None, None, :], (16, 2, 1))
        _CACHE["WV"] = wv
    return _CACHE["nc"], _CACHE["W"]


def _device_coords(score_map, trace=False):
    """Run the Bass kernel on 8 cores; return y, x arrays of shape [BS, K]
    (plus the BassKernelResults of the run)."""
    nc, W = _get_compiled()
    flat = np.ascontiguousarray(score_map.reshape(BS * K, 16, FD))
    bpc = BS // N_CORES                      # batches per core
    mpc = bpc * K                            # maps per core
    in_maps = []
    for c in range(N_CORES):
        shard = flat[c * mpc:(c + 1) * mpc].reshape(NGROUP * P, FD)
        in_maps.append({"score": shard, "wmat": W, "wvin": _CACHE["WV"]})
    res = run_bass_kernel_spmd(nc, in_maps, list(range(N_CORES)), trace=trace)

    ys = np.empty((BS, K), dtype=np.float64)
    xs = np.empty((BS, K), dtype=np.float64)
    for c in range(N_CORES):
        st = res.results[c]["stats"].astype(np.float64)   # [16, 8, 3]
        rows = np.arange(NPG)
        Z = st[2 * rows, :, 0]               # [j, g]
        S16 = st[2 * rows + 1, :, 0]
        B = st[2 * rows, :, 1]
        Xw = st[2 * rows, :, 2]
        y = (S16 + B) / Z                    # [j, g]
        x = Xw / Z
        # map (g, j) -> core-local pair index 8g + j -> (b_local, k)
        y = y.T.reshape(mpc)                 # [g, j] -> pair-major
        x = x.T.reshape(mpc)
        ys[c * bpc:(c + 1) * bpc] = y.reshape(bpc, K)
        xs[c * bpc:(c + 1) * bpc] = x.reshape(bpc, K)
    return ys, xs, res


def _exact_coords(sm64):
    """Float64 softmax expected coords for one [HO, WO] score map."""
    z = sm64 / TEMPERATURE
    z = z - z.max()
    e = np.exp(z)
    Z = e.sum()
    y = (e.sum(axis=1) * np.arange(HO)).sum() / Z
    x = (e.sum(axis=0) * np.arange(WO)).sum() / Z
    return y, x


def _soft_ceil(x):
    return x + (1.0 - 1.0 / (1.0 + np.exp(-SHARPNESS * (x - np.floor(x)))))


def _smooth_clamp(x, min_val, max_val):
    x = np.where(x < min_val,
                 min_val + SMOOTHNESS * np.tanh((x - min_val) / SMOOTHNESS), x)
    x = np.where(x > max_val,
                 max_val - SMOOTHNESS * np.tanh((max_val - x) / SMOOTHNESS), x)
    return x


def kernel(score_map, offset_map, _trace=False, _res_out=None):
    score_map = np.asarray(score_map)
    offset_map = np.asarray(offset_map)

    ys, xs, res = _device_coords(score_map, trace=_trace)
    if _res_out is not None:
        _res_out.append(res)

    coords = np.stack([ys, xs], axis=-1)          # [BS, K, 2] float64

    cc = _soft_ceil(coords)
    y_cl = _smooth_clamp(cc[..., 0], 0.0, float(HO - 1))
    x_cl = _smooth_clamp(cc[..., 1], 0.0, float(WO - 1))

    # The harness executes the reference on the same neuron jax backend,
    # where .astype(int32) rounds half-to-even (np.rint) rather than
    # truncating. Refine pairs whose clamped coords sit near a rounding
    # boundary (half-integers): the cast there is sensitive to the
    # device's bf16/f32r noise.
    fy = np.abs(y_cl - np.floor(y_cl) - 0.5)
    fx = np.abs(x_cl - np.floor(x_cl) - 0.5)
    sus = (fy < REFINE_DELTA) | (fx < REFINE_DELTA)
    for b, k in zip(*np.nonzero(sus)):
        yy, xx = _exact_coords(score_map[b, k].astype(np.float64))
        coords[b, k, 0] = yy
        coords[b, k, 1] = xx
        cc = _soft_ceil(coords[b, k])
        y_cl[b, k] = _smooth_clamp(cc[0], 0.0, float(HO - 1))
        x_cl[b, k] = _smooth_clamp(cc[1], 0.0, float(WO - 1))

    y_idx = np.rint(y_cl).astype(np.int32)
    x_idx = np.rint(x_cl).astype(np.int32)
    b_idx = np.arange(BS)[:, None]
    k_idx = np.arange(K)[None, :]
    off_y = offset_map[b_idx, 2 * k_idx, y_idx, x_idx]
    off_x = offset_map[b_idx, 2 * k_idx + 1, y_idx, x_idx]
    offset = np.stack([off_y, off_x], axis=-1)

    pts = (coords.astype(np.float32) + offset) * STRIDE
    return pts.astype(np.float32)


# revision 20
# speedup vs baseline: 1.1383x; 1.0076x over previous
"""Trainium2 Bass kernel for DifferentiableBoxParser.

Per (b, k): softmax over the 256x256 score map (T=0.1) -> expected coords
(y, x); soft-ceil + smooth-clamp; int cast; gather offsets at the resulting
index; pts = (coords + offset) * 4.

Device does the heavy part (streaming the 128 MiB score_map and computing,
per map, the softmax partial sums Z, Sy-parts, Sx). Host finishes the tiny
per-pair scalar math and the 2-element-per-pair offset gather (reading the
256 MiB offset_map on device would be pure waste: only 1024 of its elements
are needed).

Sharding: data-parallel over batch, 8 batches per core (64 maps per core).

Device layout per core: score reshaped to [1024, 4096]; group g in [0,8)
covers 8 maps; SBUF tile [128, 4096] with partition p = 16*j + s (j = map in
group, s = h-high), free f = h_low*256 + w with h = 16*s + h_low. Per 512-col
chunk q (h_low = 2q + b, b = (f%512)//256), a matmul with block-diagonal
weights accumulates into PSUM [16, 512]:
  row 2j   : colsum_j[f']  = sum_s E
  row 2j+1 : sum_s (16s + 2q) E
Finalize per group on DVE: Z = sum(row 2j); B = sum(row 2j, f' in [256,512));
S16 = sum(row 2j+1); Xw = sum(f'%256 * row 2j).
Then y = (S16 + B)/Z, x = Xw/Z on host. exp computed as exp(10*x - 40)
(softmax is shift-invariant; keeps f32 range safe).

exp output and matmul weights are bfloat16: the PE streams bf16 at 2.4 GHz
vs 1.2 GHz for f32/f32r, halving TensorE time so the whole compute pipeline
hides under the DMA stream (the kernel is HBM-bandwidth-bound). All weight
values (16s + h_low <= 255) are integers exactly representable in bf16;
PSUM accumulation stays f32. bf16 quantization of the exp values (~2^-9
relative) perturbs the expected coords by well under REFINE_DELTA; pairs
whose clamped coords land within REFINE_DELTA of a half-integer rounding
boundary are recomputed exactly on host in float64 so the non-differentiable
int cast can't flip.
"""
import sys
import numpy as np

for _p in ("/opt/trn_rl_repo", "/opt/pypackages"):
    if _p not in sys.path:
        sys.path.append(_p)

import concourse.bacc as bacc
import concourse.tile as tile
from concourse import mybir
from concourse.bass_utils import run_bass_kernel_spmd

N_CORES = 8
BS, K, HO, WO = 64, 8, 256, 256
STRIDE = 4
TEMPERATURE = 0.1
SHARPNESS = 10.0
SMOOTHNESS = 0.1
EXP_SHIFT = -42.0

NPG = 8            # maps per group
NGROUP = 8         # groups per core (8 maps/group * 8 groups = 64 maps/core)
P = 128
FD = 4096
NCHUNK = 8
MM_DT = mybir.dt.float16
REFINE_DELTA = 0.05

_CACHE = {}


def _build_nc():
    nc = bacc.Bacc(None, target_bir_lowering=False, debug=False)
    score = nc.dram_tensor("score", [NGROUP * P, FD], mybir.dt.float32,
                           kind="ExternalInput")
    wmat = nc.dram_tensor("wmat", [P, NCHUNK, 16], MM_DT, kind="ExternalInput")
    wvin = nc.dram_tensor("wvin", [16, 2, 256], mybir.dt.float32, kind="ExternalInput")
    stats = nc.dram_tensor("stats", [16, NGROUP, 3], mybir.dt.float32,
                           kind="ExternalOutput")

    with tile.TileContext(nc) as tc:
        with (
            tc.tile_pool(name="singles", bufs=1) as singles,
            tc.tile_pool(name="xin", bufs=4) as xin,
            tc.tile_pool(name="expo", bufs=3) as expo,
            tc.tile_pool(name="fin", bufs=3) as fin,
            tc.tile_pool(name="psum", bufs=4, space="PSUM") as psum_pool,
        ):
            xts = []
            for g in range(2):
                xt = xin.tile([P, FD], mybir.dt.float32)
                nc.sync.dma_start(out=xt[:], in_=score[g * P:(g + 1) * P, :])
                xts.append(xt)

            wt = singles.tile([P, NCHUNK, 16], MM_DT)
            nc.gpsimd.dma_start(out=wt[:], in_=wmat[:])
            bias_t = singles.tile([P, 1], mybir.dt.float32)
            nc.vector.memset(bias_t[:], EXP_SHIFT)
            wvec = singles.tile([16, 2, 256], mybir.dt.float32)
            nc.gpsimd.dma_start(out=wvec[:], in_=wvin[:])

            for g in range(NGROUP):
                last = g == NGROUP - 1
                # DMA in >=1 MiB chunks (max-bandwidth threshold); the last
                # group tapers so the final exp+matmul covers only 512 cols
                # after the last byte lands
                if last:
                    dsplits = [0, 2048, 3072, 4096]
                    esplits = [0, 2048, 2560, 3072, 3584, 4096]
                else:
                    dsplits = [0, 2048, 4096]
                    esplits = [0, 2048, 4096]
                if g < 2:
                    xt = xts[g]
                else:
                    xt = xin.tile([P, FD], mybir.dt.float32)
                    for d0, d1 in zip(dsplits[:-1], dsplits[1:]):
                        nc.sync.dma_start(out=xt[:, d0:d1],
                                          in_=score[g * P:(g + 1) * P, d0:d1])
                ps = psum_pool.tile([16, 512], mybir.dt.float32)
                cw = 512
                # one et tile PER exp chunk: a shared per-group tile makes
                # the tile tracker serialize exp(chunk k+1) behind the
                # matmuls still reading chunk k (whole-tile WAR), killing
                # the ACT/PE overlap
                for ci, (a0, a1) in enumerate(zip(esplits[:-1], esplits[1:])):
                    tg = f"lt{ci}" if last else f"et{ci}"
                    et = expo.tile([P, a1 - a0], MM_DT, tag=tg)
                    nc.scalar.activation(out=et[:],
                                         in_=xt[:, a0:a1],
                                         func=mybir.ActivationFunctionType.Exp,
                                         bias=bias_t[:], scale=1.0 / TEMPERATURE)
                    for q in range(a0 // cw, a1 // cw):
                        nc.tensor.matmul(
                            ps[:, 0:cw],
                            wt[:, q, :],
                            et[:, q * cw - a0:(q + 1) * cw - a0],
                            start=(q == 0), stop=(q == NCHUNK - 1),
                        )

                tmp = fin.tile([16, 512], mybir.dt.float32)
                wv = wvec.rearrange("p a b -> p (a b)")
                if last:
                    # final drain: DVE does Z then fused Xw (mul+accum in
                    # one op) while ACT — idle after its last exp — does B
                    # via Copy+accum_out; three independent stats DMAs go
                    # out on two rings as each stat completes
                    obx = fin.tile([16, 1], mybir.dt.float32, tag="obx")
                    obz = fin.tile([16, 1], mybir.dt.float32, tag="obz")
                    obb = fin.tile([16, 1], mybir.dt.float32, tag="obb")
                    nc.vector.reduce_sum(obz[:], ps[:],
                                         axis=mybir.AxisListType.X)
                    nc.vector.scalar_tensor_tensor(
                        out=tmp[:], in0=ps[:], scalar=1.0, in1=wv[:],
                        op0=mybir.AluOpType.mult, op1=mybir.AluOpType.mult,
                        accum_out=obx[:])
                    junk = fin.tile([16, 256], mybir.dt.float32, tag="junk")
                    nc.scalar.activation(out=junk[:], in_=ps[:, 256:512],
                                         func=mybir.ActivationFunctionType.Copy,
                                         accum_out=obb[:])
                    nc.sync.dma_start(out=stats[:, g, 0:1], in_=obz[:])
                    nc.sync.dma_start(out=stats[:, g, 1:2], in_=obb[:])
                    # Xw's DMA rides the gpsimd SWDGE so it doesn't queue
                    # behind the two sync-ring descriptor gens
                    nc.gpsimd.dma_start(out=stats[:, g, 2:3], in_=obx[:])
                else:
                    ob = fin.tile([16, 3], mybir.dt.float32, tag="ob")
                    nc.vector.reduce_sum(ob[:, 0:1], ps[:, 0:cw],
                                         axis=mybir.AxisListType.X)
                    nc.vector.reduce_sum(ob[:, 1:2], ps[:, 256:512],
                                         axis=mybir.AxisListType.X)
                    nc.vector.scalar_tensor_tensor(
                        out=tmp[:, 0:cw], in0=ps[:, 0:cw], scalar=1.0, in1=wv[:],
                        op0=mybir.AluOpType.mult, op1=mybir.AluOpType.mult,
                        accum_out=ob[:, 2:3])
                    # per-group stats write on the otherwise-idle gpsimd
                    # SWDGE so it never head-of-line blocks the sync ring
                    # streaming score
                    nc.gpsimd.dma_start(out=stats[:, g, :], in_=ob[:])

    nc.compile()
    return nc


def _weights():
    W = np.zeros((P, NCHUNK, 16), dtype=np.float32)
    s = np.arange(16)
    for j in range(NPG):
        W[16 * j + s, :, 2 * j] = 1.0
        for q in range(NCHUNK):          # 512-wide chunks: h_low = 2q + b
            W[16 * j + s, q, 2 * j + 1] = (16 * s + 2 * q).astype(np.float32)
    return W.astype(np.float16)


def _get_compiled():
    if "nc" not in _CACHE:
        _CACHE["nc"] = _build_nc()
        _CACHE["W"] = _weights()
        wv = np.tile(np.arange(256, dtype=np.float32)[None, None, :], (16, 2, 1))
        _CACHE["WV"] = wv
    return _CACHE["nc"], _CACHE["W"]


def _device_coords(score_map, trace=False):
    """Run the Bass kernel on 8 cores; return y, x arrays of shape [BS, K]
    (plus the BassKernelResults of the run)."""
    nc, W = _get_compiled()
    flat = np.ascontiguousarray(score_map.reshape(BS * K, 16, FD))
    bpc = BS // N_CORES                      # batches per core
    mpc = bpc * K                            # maps per core
    in_maps = []
    for c in range(N_CORES):
        shard = flat[c * mpc:(c + 1) * mpc].reshape(NGROUP * P, FD)
        in_maps.append({"score": shard, "wmat": W, "wvin": _CACHE["WV"]})
    res = run_bass_kernel_spmd(nc, in_maps, list(range(N_CORES)), trace=trace)

    ys = np.empty((BS, K), dtype=np.float64)
    xs = np.empty((BS, K), dtype=np.float64)
    for c in range(N_CORES):
        st = res.results[c]["stats"].astype(np.float64)   # [16, 8, 3]
        rows = np.arange(NPG)
        Z = st[2 * rows, :, 0]               # [j, g]
        S16 = st[2 * rows + 1, :, 0]
        B = st[2 * rows, :, 1]
        Xw = st[2 * rows, :, 2]
        y = (S16 + B) / Z                    # [j, g]
        x = Xw / Z
        # map (g, j) -> core-local pair index 8g + j -> (b_local, k)
        y = y.T.reshape(mpc)                 # [g, j] -> pair-major
        x = x.T.reshape(mpc)
        ys[c * bpc:(c + 1) * bpc] = y.reshape(bpc, K)
        xs[c * bpc:(c + 1) * bpc] = x.reshape(bpc, K)
    return ys, xs, res


def _exact_coords(sm64):
    """Float64 softmax expected coords for one [HO, WO] score map."""
    z = sm64 / TEMPERATURE
    z = z - z.max()
    e = np.exp(z)
    Z = e.sum()
    y = (e.sum(axis=1) * np.arange(HO)).sum() / Z
    x = (e.sum(axis=0) * np.arange(WO)).sum() / Z
    return y, x


def _soft_ceil(x):
    return x + (1.0 - 1.0 / (1.0 + np.exp(-SHARPNESS * (x - np.floor(x)))))


def _smooth_clamp(x, min_val, max_val):
    x = np.where(x < min_val,
                 min_val + SMOOTHNESS * np.tanh((x - min_val) / SMOOTHNESS), x)
    x = np.where(x > max_val,
                 max_val - SMOOTHNESS * np.tanh((max_val - x) / SMOOTHNESS), x)
    return x


def kernel(score_map, offset_map, _trace=False, _res_out=None):
    score_map = np.asarray(score_map)
    offset_map = np.asarray(offset_map)

    ys, xs, res = _device_coords(score_map, trace=_trace)
    if _res_out is not None:
        _res_out.append(res)

    coords = np.stack([ys, xs], axis=-1)          # [BS, K, 2] float64

    cc = _soft_ceil(coords)
    y_cl = _smooth_clamp(cc[..., 0], 0.0, float(HO - 1))
    x_cl = _smooth_clamp(cc[..., 1], 0.0, float(WO - 1))

    # The harness executes the reference on the same neuron jax backend,
    # where .astype(int32) rounds half-to-even (np.rint) rather than
    # truncating. Refine pairs whose clamped coords sit near a rounding
    # boundary (half-integers): the cast there is sensitive to the
    # device's bf16/f32r noise.
    fy = np.abs(y_cl - np.floor(y_cl) - 0.5)
    fx = np.abs(x_cl - np.floor(x_cl) - 0.5)
    sus = (fy < REFINE_DELTA) | (fx < REFINE_DELTA)
    for b, k in zip(*np.nonzero(sus)):
        yy, xx = _exact_coords(score_map[b, k].astype(np.float64))
        coords[b, k, 0] = yy
        coords[b, k, 1] = xx
        cc = _soft_ceil(coords[b, k])
        y_cl[b, k] = _smooth_clamp(cc[0], 0.0, float(HO - 1))
        x_cl[b, k] = _smooth_clamp(cc[1], 0.0, float(WO - 1))

    y_idx = np.rint(y_cl).astype(np.int32)
    x_idx = np.rint(x_cl).astype(np.int32)
    b_idx = np.arange(BS)[:, None]
    k_idx = np.arange(K)[None, :]
    off_y = offset_map[b_idx, 2 * k_idx, y_idx, x_idx]
    off_x = offset_map[b_idx, 2 * k_idx + 1, y_idx, x_idx]
    offset = np.stack([off_y, off_x], axis=-1)

    pts = (coords.astype(np.float32) + offset) * STRIDE
    return pts.astype(np.float32)


# revision 21
# speedup vs baseline: 1.1471x; 1.0077x over previous
"""Trainium2 Bass kernel for DifferentiableBoxParser.

Per (b, k): softmax over the 256x256 score map (T=0.1) -> expected coords
(y, x); soft-ceil + smooth-clamp; int cast; gather offsets at the resulting
index; pts = (coords + offset) * 4.

Device does the heavy part (streaming the 128 MiB score_map and computing,
per map, the softmax partial sums Z, Sy-parts, Sx). Host finishes the tiny
per-pair scalar math and the 2-element-per-pair offset gather (reading the
256 MiB offset_map on device would be pure waste: only 1024 of its elements
are needed).

Sharding: data-parallel over batch, 8 batches per core (64 maps per core).

Device layout per core: score reshaped to [1024, 4096]; group g in [0,8)
covers 8 maps; SBUF tile [128, 4096] with partition p = 16*j + s (j = map in
group, s = h-high), free f = h_low*256 + w with h = 16*s + h_low. Per 512-col
chunk q (h_low = 2q + b, b = (f%512)//256), a matmul with block-diagonal
weights accumulates into PSUM [16, 512]:
  row 2j   : colsum_j[f']  = sum_s E
  row 2j+1 : sum_s (16s + 2q) E
Finalize per group on DVE: Z = sum(row 2j); B = sum(row 2j, f' in [256,512));
S16 = sum(row 2j+1); Xw = sum(f'%256 * row 2j).
Then y = (S16 + B)/Z, x = Xw/Z on host. exp computed as exp(10*x - 40)
(softmax is shift-invariant; keeps f32 range safe).

exp output and matmul weights are bfloat16: the PE streams bf16 at 2.4 GHz
vs 1.2 GHz for f32/f32r, halving TensorE time so the whole compute pipeline
hides under the DMA stream (the kernel is HBM-bandwidth-bound). All weight
values (16s + h_low <= 255) are integers exactly representable in bf16;
PSUM accumulation stays f32. bf16 quantization of the exp values (~2^-9
relative) perturbs the expected coords by well under REFINE_DELTA; pairs
whose clamped coords land within REFINE_DELTA of a half-integer rounding
boundary are recomputed exactly on host in float64 so the non-differentiable
int cast can't flip.
"""
import sys
import numpy as np

for _p in ("/opt/trn_rl_repo", "/opt/pypackages"):
    if _p not in sys.path:
        sys.path.append(_p)

import concourse.bacc as bacc
import concourse.tile as tile
from concourse import mybir
from concourse.bass_utils import run_bass_kernel_spmd

N_CORES = 8
BS, K, HO, WO = 64, 8, 256, 256
STRIDE = 4
TEMPERATURE = 0.1
SHARPNESS = 10.0
SMOOTHNESS = 0.1
EXP_SHIFT = -42.0

NPG = 8            # maps per group
NGROUP = 8         # groups per core (8 maps/group * 8 groups = 64 maps/core)
P = 128
FD = 4096
NCHUNK = 8
MM_DT = mybir.dt.float16
REFINE_DELTA = 0.05

_CACHE = {}


def _build_nc():
    nc = bacc.Bacc(None, target_bir_lowering=False, debug=False)
    score = nc.dram_tensor("score", [NGROUP * P, FD], mybir.dt.float32,
                           kind="ExternalInput")
    wmat = nc.dram_tensor("wmat", [P, NCHUNK, 16], MM_DT, kind="ExternalInput")
    wvin = nc.dram_tensor("wvin", [16, 2, 256], mybir.dt.float32, kind="ExternalInput")
    stats = nc.dram_tensor("stats", [16, NGROUP, 3], mybir.dt.float32,
                           kind="ExternalOutput")

    with tile.TileContext(nc) as tc:
        with (
            tc.tile_pool(name="singles", bufs=1) as singles,
            tc.tile_pool(name="xin", bufs=4) as xin,
            tc.tile_pool(name="expo", bufs=3) as expo,
            tc.tile_pool(name="fin", bufs=3) as fin,
            tc.tile_pool(name="psum", bufs=4, space="PSUM") as psum_pool,
        ):
            xts = []
            for g in range(2):
                xt = xin.tile([P, FD], mybir.dt.float32)
                nc.sync.dma_start(out=xt[:], in_=score[g * P:(g + 1) * P, :])
                xts.append(xt)

            wt = singles.tile([P, NCHUNK, 16], MM_DT)
            nc.gpsimd.dma_start(out=wt[:], in_=wmat[:])
            bias_t = singles.tile([P, 1], mybir.dt.float32)
            nc.vector.memset(bias_t[:], EXP_SHIFT)
            wvec = singles.tile([16, 2, 256], mybir.dt.float32)
            nc.gpsimd.dma_start(out=wvec[:], in_=wvin[:])

            for g in range(NGROUP):
                last = g == NGROUP - 1
                # DMA in >=1 MiB chunks (max-bandwidth threshold); the last
                # group tapers so the final exp+matmul covers only 512 cols
                # after the last byte lands
                if last:
                    dsplits = [0, 1024, 2048, 2560, 3072, 3584, 4096]
                    esplits = [0, 1024, 2048, 2560, 3072, 3584, 4096]
                else:
                    dsplits = [0, 2048, 4096]
                    esplits = [0, 2048, 4096]
                if g < 2:
                    xt = xts[g]
                else:
                    xt = xin.tile([P, FD], mybir.dt.float32)
                    for d0, d1 in zip(dsplits[:-1], dsplits[1:]):
                        nc.sync.dma_start(out=xt[:, d0:d1],
                                          in_=score[g * P:(g + 1) * P, d0:d1])
                ps = psum_pool.tile([16, 512], mybir.dt.float32)
                cw = 512
                # one et tile PER exp chunk: a shared per-group tile makes
                # the tile tracker serialize exp(chunk k+1) behind the
                # matmuls still reading chunk k (whole-tile WAR), killing
                # the ACT/PE overlap
                for ci, (a0, a1) in enumerate(zip(esplits[:-1], esplits[1:])):
                    tg = f"lt{ci}" if last else f"et{ci}"
                    et = expo.tile([P, a1 - a0], MM_DT, tag=tg)
                    nc.scalar.activation(out=et[:],
                                         in_=xt[:, a0:a1],
                                         func=mybir.ActivationFunctionType.Exp,
                                         bias=bias_t[:], scale=1.0 / TEMPERATURE)
                    for q in range(a0 // cw, a1 // cw):
                        nc.tensor.matmul(
                            ps[:, 0:cw],
                            wt[:, q, :],
                            et[:, q * cw - a0:(q + 1) * cw - a0],
                            start=(q == 0), stop=(q == NCHUNK - 1),
                        )

                tmp = fin.tile([16, 512], mybir.dt.float32)
                wv = wvec.rearrange("p a b -> p (a b)")
                if last:
                    # final drain: DVE does Z then fused Xw (mul+accum in
                    # one op) while ACT — idle after its last exp — does B
                    # via Copy+accum_out; three independent stats DMAs go
                    # out on two rings as each stat completes
                    obx = fin.tile([16, 1], mybir.dt.float32, tag="obx")
                    obz = fin.tile([16, 1], mybir.dt.float32, tag="obz")
                    obb = fin.tile([16, 1], mybir.dt.float32, tag="obb")
                    nc.vector.reduce_sum(obz[:], ps[:],
                                         axis=mybir.AxisListType.X)
                    nc.vector.scalar_tensor_tensor(
                        out=tmp[:], in0=ps[:], scalar=1.0, in1=wv[:],
                        op0=mybir.AluOpType.mult, op1=mybir.AluOpType.mult,
                        accum_out=obx[:])
                    junk = fin.tile([16, 256], mybir.dt.float32, tag="junk")
                    nc.scalar.activation(out=junk[:], in_=ps[:, 256:512],
                                         func=mybir.ActivationFunctionType.Copy,
                                         accum_out=obb[:])
                    nc.sync.dma_start(out=stats[:, g, 0:1], in_=obz[:])
                    nc.sync.dma_start(out=stats[:, g, 1:2], in_=obb[:])
                    # Xw's DMA rides the gpsimd SWDGE so it doesn't queue
                    # behind the two sync-ring descriptor gens
                    nc.gpsimd.dma_start(out=stats[:, g, 2:3], in_=obx[:])
                else:
                    ob = fin.tile([16, 3], mybir.dt.float32, tag="ob")
                    nc.vector.reduce_sum(ob[:, 0:1], ps[:, 0:cw],
                                         axis=mybir.AxisListType.X)
                    nc.vector.reduce_sum(ob[:, 1:2], ps[:, 256:512],
                                         axis=mybir.AxisListType.X)
                    nc.vector.scalar_tensor_tensor(
                        out=tmp[:, 0:cw], in0=ps[:, 0:cw], scalar=1.0, in1=wv[:],
                        op0=mybir.AluOpType.mult, op1=mybir.AluOpType.mult,
                        accum_out=ob[:, 2:3])
                    # per-group stats write on the otherwise-idle gpsimd
                    # SWDGE so it never head-of-line blocks the sync ring
                    # streaming score
                    nc.gpsimd.dma_start(out=stats[:, g, :], in_=ob[:])

    nc.compile()
    return nc


def _weights():
    W = np.zeros((P, NCHUNK, 16), dtype=np.float32)
    s = np.arange(16)
    for j in range(NPG):
        W[16 * j + s, :, 2 * j] = 1.0
        for q in range(NCHUNK):          # 512-wide chunks: h_low = 2q + b
            W[16 * j + s, q, 2 * j + 1] = (16 * s + 2 * q).astype(np.float32)
    return W.astype(np.float16)


def _get_compiled():
    if "nc" not in _CACHE:
        _CACHE["nc"] = _build_nc()
        _CACHE["W"] = _weights()
        wv = np.tile(np.arange(256, dtype=np.float32)[None, None, :], (16, 2, 1))
        _CACHE["WV"] = wv
    return _CACHE["nc"], _CACHE["W"]


def _device_coords(score_map, trace=False):
    """Run the Bass kernel on 8 cores; return y, x arrays of shape [BS, K]
    (plus the BassKernelResults of the run)."""
    nc, W = _get_compiled()
    flat = np.ascontiguousarray(score_map.reshape(BS * K, 16, FD))
    bpc = BS // N_CORES                      # batches per core
    mpc = bpc * K                            # maps per core
    in_maps = []
    for c in range(N_CORES):
        shard = flat[c * mpc:(c + 1) * mpc].reshape(NGROUP * P, FD)
        in_maps.append({"score": shard, "wmat": W, "wvin": _CACHE["WV"]})
    res = run_bass_kernel_spmd(nc, in_maps, list(range(N_CORES)), trace=trace)

    ys = np.empty((BS, K), dtype=np.float64)
    xs = np.empty((BS, K), dtype=np.float64)
    for c in range(N_CORES):
        st = res.results[c]["stats"].astype(np.float64)   # [16, 8, 3]
        rows = np.arange(NPG)
        Z = st[2 * rows, :, 0]               # [j, g]
        S16 = st[2 * rows + 1, :, 0]
        B = st[2 * rows, :, 1]
        Xw = st[2 * rows, :, 2]
        y = (S16 + B) / Z                    # [j, g]
        x = Xw / Z
        # map (g, j) -> core-local pair index 8g + j -> (b_local, k)
        y = y.T.reshape(mpc)                 # [g, j] -> pair-major
        x = x.T.reshape(mpc)
        ys[c * bpc:(c + 1) * bpc] = y.reshape(bpc, K)
        xs[c * bpc:(c + 1) * bpc] = x.reshape(bpc, K)
    return ys, xs, res


def _exact_coords(sm64):
    """Float64 softmax expected coords for one [HO, WO] score map."""
    z = sm64 / TEMPERATURE
    z = z - z.max()
    e = np.exp(z)
    Z = e.sum()
    y = (e.sum(axis=1) * np.arange(HO)).sum() / Z
    x = (e.sum(axis=0) * np.arange(WO)).sum() / Z
    return y, x


def _soft_ceil(x):
    return x + (1.0 - 1.0 / (1.0 + np.exp(-SHARPNESS * (x - np.floor(x)))))


def _smooth_clamp(x, min_val, max_val):
    x = np.where(x < min_val,
                 min_val + SMOOTHNESS * np.tanh((x - min_val) / SMOOTHNESS), x)
    x = np.where(x > max_val,
                 max_val - SMOOTHNESS * np.tanh((max_val - x) / SMOOTHNESS), x)
    return x


def kernel(score_map, offset_map, _trace=False, _res_out=None):
    score_map = np.asarray(score_map)
    offset_map = np.asarray(offset_map)

    ys, xs, res = _device_coords(score_map, trace=_trace)
    if _res_out is not None:
        _res_out.append(res)

    coords = np.stack([ys, xs], axis=-1)          # [BS, K, 2] float64

    cc = _soft_ceil(coords)
    y_cl = _smooth_clamp(cc[..., 0], 0.0, float(HO - 1))
    x_cl = _smooth_clamp(cc[..., 1], 0.0, float(WO - 1))

    # The harness executes the reference on the same neuron jax backend,
    # where .astype(int32) rounds half-to-even (np.rint) rather than
    # truncating. Refine pairs whose clamped coords sit near a rounding
    # boundary (half-integers): the cast there is sensitive to the
    # device's bf16/f32r noise.
    fy = np.abs(y_cl - np.floor(y_cl) - 0.5)
    fx = np.abs(x_cl - np.floor(x_cl) - 0.5)
    sus = (fy < REFINE_DELTA) | (fx < REFINE_DELTA)
    for b, k in zip(*np.nonzero(sus)):
        yy, xx = _exact_coords(score_map[b, k].astype(np.float64))
        coords[b, k, 0] = yy
        coords[b, k, 1] = xx
        cc = _soft_ceil(coords[b, k])
        y_cl[b, k] = _smooth_clamp(cc[0], 0.0, float(HO - 1))
        x_cl[b, k] = _smooth_clamp(cc[1], 0.0, float(WO - 1))

    y_idx = np.rint(y_cl).astype(np.int32)
    x_idx = np.rint(x_cl).astype(np.int32)
    b_idx = np.arange(BS)[:, None]
    k_idx = np.arange(K)[None, :]
    off_y = offset_map[b_idx, 2 * k_idx, y_idx, x_idx]
    off_x = offset_map[b_idx, 2 * k_idx + 1, y_idx, x_idx]
    offset = np.stack([off_y, off_x], axis=-1)

    pts = (coords.astype(np.float32) + offset) * STRIDE
    return pts.astype(np.float32)


# revision 26
# speedup vs baseline: 1.1572x; 1.0088x over previous
"""Trainium2 Bass kernel for DifferentiableBoxParser.

Per (b, k): softmax over the 256x256 score map (T=0.1) -> expected coords
(y, x); soft-ceil + smooth-clamp; int cast; gather offsets at the resulting
index; pts = (coords + offset) * 4.

Device does the heavy part (streaming the 128 MiB score_map and computing,
per map, the softmax partial sums Z, Sy-parts, Sx). Host finishes the tiny
per-pair scalar math and the 2-element-per-pair offset gather (reading the
256 MiB offset_map on device would be pure waste: only 1024 of its elements
are needed).

Sharding: data-parallel over batch, 8 batches per core (64 maps per core).

Device layout per core: score reshaped to [1024, 4096]; group g in [0,8)
covers 8 maps; SBUF tile [128, 4096] with partition p = 16*j + s (j = map in
group, s = h-high), free f = h_low*256 + w with h = 16*s + h_low. Per 512-col
chunk q (h_low = 2q + b, b = (f%512)//256), a matmul with block-diagonal
weights accumulates into PSUM [16, 512]:
  row 2j   : colsum_j[f']  = sum_s E
  row 2j+1 : sum_s (16s + 2q) E
Finalize per group on DVE: Z = sum(row 2j); B = sum(row 2j, f' in [256,512));
S16 = sum(row 2j+1); Xw = sum(f'%256 * row 2j).
Then y = (S16 + B)/Z, x = Xw/Z on host. exp computed as exp(10*x - 40)
(softmax is shift-invariant; keeps f32 range safe).

exp output and matmul weights are bfloat16: the PE streams bf16 at 2.4 GHz
vs 1.2 GHz for f32/f32r, halving TensorE time so the whole compute pipeline
hides under the DMA stream (the kernel is HBM-bandwidth-bound). All weight
values (16s + h_low <= 255) are integers exactly representable in bf16;
PSUM accumulation stays f32. bf16 quantization of the exp values (~2^-9
relative) perturbs the expected coords by well under REFINE_DELTA; pairs
whose clamped coords land within REFINE_DELTA of a half-integer rounding
boundary are recomputed exactly on host in float64 so the non-differentiable
int cast can't flip.
"""
import sys
import numpy as np

for _p in ("/opt/trn_rl_repo", "/opt/pypackages"):
    if _p not in sys.path:
        sys.path.append(_p)

import concourse.bacc as bacc
import concourse.tile as tile
from concourse import mybir
from concourse.bass_utils import run_bass_kernel_spmd

N_CORES = 8
BS, K, HO, WO = 64, 8, 256, 256
STRIDE = 4
TEMPERATURE = 0.1
SHARPNESS = 10.0
SMOOTHNESS = 0.1
EXP_SHIFT = -42.0

NPG = 8            # maps per group
NGROUP = 8         # groups per core (8 maps/group * 8 groups = 64 maps/core)
P = 128
FD = 4096
NCHUNK = 8
MM_DT = mybir.dt.float16
REFINE_DELTA = 0.05

_CACHE = {}


def _build_nc():
    nc = bacc.Bacc(None, target_bir_lowering=False, debug=False)
    score = nc.dram_tensor("score", [NGROUP * P, FD], mybir.dt.float32,
                           kind="ExternalInput")
    wmat = nc.dram_tensor("wmat", [P, NCHUNK, 16], MM_DT, kind="ExternalInput")
    wvin = nc.dram_tensor("wvin", [16, 2, 256], mybir.dt.float32, kind="ExternalInput")
    stats = nc.dram_tensor("stats", [16, NGROUP + 1, 3], mybir.dt.float32,
                           kind="ExternalOutput")

    with tile.TileContext(nc) as tc:
        with (
            tc.tile_pool(name="singles", bufs=1) as singles,
            tc.tile_pool(name="xin", bufs=4) as xin,
            tc.tile_pool(name="expo", bufs=3) as expo,
            tc.tile_pool(name="fin", bufs=3) as fin,
            tc.tile_pool(name="psum", bufs=4, space="PSUM") as psum_pool,
        ):
            xts = []
            for g in range(2):
                xt = xin.tile([P, FD], mybir.dt.float32)
                nc.sync.dma_start(out=xt[:], in_=score[g * P:(g + 1) * P, :])
                xts.append(xt)

            wt = singles.tile([P, NCHUNK, 16], MM_DT)
            nc.gpsimd.dma_start(out=wt[:], in_=wmat[:])
            bias_t = singles.tile([P, 1], mybir.dt.float32)
            nc.vector.memset(bias_t[:], EXP_SHIFT)
            wvec = singles.tile([16, 2, 256], mybir.dt.float32)
            nc.gpsimd.dma_start(out=wvec[:], in_=wvin[:])

            for g in range(NGROUP):
                last = g == NGROUP - 1
                # DMA in >=1 MiB chunks (max-bandwidth threshold); the last
                # group tapers so the final exp+matmul covers only 512 cols
                # after the last byte lands
                if last:
                    dsplits = [0, 1024, 2048, 2560, 3072, 3584, 4096]
                    esplits = dsplits
                elif g == NGROUP - 2:
                    # tapered too, so ACT enters the last group with no
                    # exp backlog
                    dsplits = [0, 2048, 3072, 4096]
                    esplits = [0, 2048, 3072, 3584, 4096]
                else:
                    dsplits = [0, 2048, 4096]
                    esplits = [0, 2048, 4096]
                if g < 2:
                    xt = xts[g]
                else:
                    xt = xin.tile([P, FD], mybir.dt.float32)
                    for d0, d1 in zip(dsplits[:-1], dsplits[1:]):
                        nc.sync.dma_start(out=xt[:, d0:d1],
                                          in_=score[g * P:(g + 1) * P, d0:d1])
                cw = 512
                # the last group accumulates chunks 0-5 and 6-7 into two
                # separate psum tiles: the 0-5 finalize runs hidden while
                # chunks 6-7 still stream; host sums the two stat halves
                if last:
                    psa = psum_pool.tile([16, 512], mybir.dt.float32, tag="psa")
                    psb = psum_pool.tile([16, 512], mybir.dt.float32, tag="psb")
                    pslim = [(0, 5), (6, NCHUNK - 1)]
                else:
                    psa = psum_pool.tile([16, 512], mybir.dt.float32, tag="psa")
                    psb = None
                    pslim = [(0, NCHUNK - 1)]
                # one et tile PER exp chunk: a shared per-group tile makes
                # the tile tracker serialize exp(chunk k+1) behind the
                # matmuls still reading chunk k (whole-tile WAR), killing
                # the ACT/PE overlap. Tag by (offset, width): tiles sharing
                # a tag ring must be the same size.
                for a0, a1 in zip(esplits[:-1], esplits[1:]):
                    et = expo.tile([P, a1 - a0], MM_DT, tag=f"c{a0}_{a1 - a0}")
                    nc.scalar.activation(out=et[:],
                                         in_=xt[:, a0:a1],
                                         func=mybir.ActivationFunctionType.Exp,
                                         bias=bias_t[:], scale=1.0 / TEMPERATURE)
                    for q in range(a0 // cw, a1 // cw):
                        ps = psb if (last and q >= 6) else psa
                        q0, q1 = pslim[1] if (last and q >= 6) else pslim[0]
                        nc.tensor.matmul(
                            ps[:, 0:cw],
                            wt[:, q, :],
                            et[:, q * cw - a0:(q + 1) * cw - a0],
                            start=(q == q0), stop=(q == q1),
                        )

                wv = wvec.rearrange("p a b -> p (a b)")
                if last:
                    # finalize each psum half as it stops: DVE does Z then
                    # fused Xw (mul+accum in one op) while ACT — idle after
                    # its exps — does B via Copy+accum_out; half A's stats
                    # go to slot g, half B's to slot g+1 (host sums them)
                    for half, (pst, slot) in enumerate([(psa, g), (psb, g + 1)]):
                        tmp = fin.tile([16, 512], mybir.dt.float32,
                                       tag=f"tmp{half}")
                        obx = fin.tile([16, 1], mybir.dt.float32, tag=f"obx{half}")
                        obz = fin.tile([16, 1], mybir.dt.float32, tag=f"obz{half}")
                        obb = fin.tile([16, 1], mybir.dt.float32, tag=f"obb{half}")
                        nc.vector.reduce_sum(obz[:], pst[:],
                                             axis=mybir.AxisListType.X)
                        nc.vector.scalar_tensor_tensor(
                            out=tmp[:], in0=pst[:], scalar=1.0, in1=wv[:],
                            op0=mybir.AluOpType.mult, op1=mybir.AluOpType.mult,
                            accum_out=obx[:])
                        junk = fin.tile([16, 256], mybir.dt.float32,
                                        tag=f"junk{half}")
                        nc.scalar.activation(out=junk[:], in_=pst[:, 256:512],
                                             func=mybir.ActivationFunctionType.Copy,
                                             accum_out=obb[:])
                        nc.sync.dma_start(out=stats[:, slot, 0:1], in_=obz[:])
                        nc.sync.dma_start(out=stats[:, slot, 1:2], in_=obb[:])
                        # Xw's DMA rides the gpsimd SWDGE so it doesn't
                        # queue behind the sync-ring descriptor gens
                        nc.gpsimd.dma_start(out=stats[:, slot, 2:3], in_=obx[:])
                else:
                    tmp = fin.tile([16, 512], mybir.dt.float32)
                    ob = fin.tile([16, 3], mybir.dt.float32, tag="ob")
                    nc.vector.reduce_sum(ob[:, 0:1], psa[:, 0:cw],
                                         axis=mybir.AxisListType.X)
                    nc.vector.reduce_sum(ob[:, 1:2], psa[:, 256:512],
                                         axis=mybir.AxisListType.X)
                    nc.vector.scalar_tensor_tensor(
                        out=tmp[:, 0:cw], in0=psa[:, 0:cw], scalar=1.0, in1=wv[:],
                        op0=mybir.AluOpType.mult, op1=mybir.AluOpType.mult,
                        accum_out=ob[:, 2:3])
                    # per-group stats write on the otherwise-idle gpsimd
                    # SWDGE so it never head-of-line blocks the sync ring
                    # streaming score
                    nc.gpsimd.dma_start(out=stats[:, g, :], in_=ob[:])

    nc.compile()
    return nc


def _weights():
    W = np.zeros((P, NCHUNK, 16), dtype=np.float32)
    s = np.arange(16)
    for j in range(NPG):
        W[16 * j + s, :, 2 * j] = 1.0
        for q in range(NCHUNK):          # 512-wide chunks: h_low = 2q + b
            W[16 * j + s, q, 2 * j + 1] = (16 * s + 2 * q).astype(np.float32)
    return W.astype(np.float16)


def _get_compiled():
    if "nc" not in _CACHE:
        _CACHE["nc"] = _build_nc()
        _CACHE["W"] = _weights()
        wv = np.tile(np.arange(256, dtype=np.float32)[None, None, :], (16, 2, 1))
        _CACHE["WV"] = wv
    return _CACHE["nc"], _CACHE["W"]


def _device_coords(score_map, trace=False):
    """Run the Bass kernel on 8 cores; return y, x arrays of shape [BS, K]
    (plus the BassKernelResults of the run)."""
    nc, W = _get_compiled()
    flat = np.ascontiguousarray(score_map.reshape(BS * K, 16, FD))
    bpc = BS // N_CORES                      # batches per core
    mpc = bpc * K                            # maps per core
    in_maps = []
    for c in range(N_CORES):
        shard = flat[c * mpc:(c + 1) * mpc].reshape(NGROUP * P, FD)
        in_maps.append({"score": shard, "wmat": W, "wvin": _CACHE["WV"]})
    res = run_bass_kernel_spmd(nc, in_maps, list(range(N_CORES)), trace=trace)

    ys = np.empty((BS, K), dtype=np.float64)
    xs = np.empty((BS, K), dtype=np.float64)
    for c in range(N_CORES):
        st = res.results[c]["stats"].astype(np.float64)   # [16, 9, 3]
        st[:, NGROUP - 1, :] += st[:, NGROUP, :]   # last group's two halves
        st = st[:, :NGROUP, :]
        rows = np.arange(NPG)
        Z = st[2 * rows, :, 0]               # [j, g]
        S16 = st[2 * rows + 1, :, 0]
        B = st[2 * rows, :, 1]
        Xw = st[2 * rows, :, 2]
        y = (S16 + B) / Z                    # [j, g]
        x = Xw / Z
        # map (g, j) -> core-local pair index 8g + j -> (b_local, k)
        y = y.T.reshape(mpc)                 # [g, j] -> pair-major
        x = x.T.reshape(mpc)
        ys[c * bpc:(c + 1) * bpc] = y.reshape(bpc, K)
        xs[c * bpc:(c + 1) * bpc] = x.reshape(bpc, K)
    return ys, xs, res


def _exact_coords(sm64):
    """Float64 softmax expected coords for one [HO, WO] score map."""
    z = sm64 / TEMPERATURE
    z = z - z.max()
    e = np.exp(z)
    Z = e.sum()
    y = (e.sum(axis=1) * np.arange(HO)).sum() / Z
    x = (e.sum(axis=0) * np.arange(WO)).sum() / Z
    return y, x


def _soft_ceil(x):
    return x + (1.0 - 1.0 / (1.0 + np.exp(-SHARPNESS * (x - np.floor(x)))))


def _smooth_clamp(x, min_val, max_val):
    x = np.where(x < min_val,
                 min_val + SMOOTHNESS * np.tanh((x - min_val) / SMOOTHNESS), x)
    x = np.where(x > max_val,
                 max_val - SMOOTHNESS * np.tanh((max_val - x) / SMOOTHNESS), x)
    return x


def kernel(score_map, offset_map, _trace=False, _res_out=None):
    score_map = np.asarray(score_map)
    offset_map = np.asarray(offset_map)

    ys, xs, res = _device_coords(score_map, trace=_trace)
    if _res_out is not None:
        _res_out.append(res)

    coords = np.stack([ys, xs], axis=-1)          # [BS, K, 2] float64

    cc = _soft_ceil(coords)
    y_cl = _smooth_clamp(cc[..., 0], 0.0, float(HO - 1))
    x_cl = _smooth_clamp(cc[..., 1], 0.0, float(WO - 1))

    # The harness executes the reference on the same neuron jax backend,
    # where .astype(int32) rounds half-to-even (np.rint) rather than
    # truncating. Refine pairs whose clamped coords sit near a rounding
    # boundary (half-integers): the cast there is sensitive to the
    # device's bf16/f32r noise.
    fy = np.abs(y_cl - np.floor(y_cl) - 0.5)
    fx = np.abs(x_cl - np.floor(x_cl) - 0.5)
    sus = (fy < REFINE_DELTA) | (fx < REFINE_DELTA)
    for b, k in zip(*np.nonzero(sus)):
        yy, xx = _exact_coords(score_map[b, k].astype(np.float64))
        coords[b, k, 0] = yy
        coords[b, k, 1] = xx
        cc = _soft_ceil(coords[b, k])
        y_cl[b, k] = _smooth_clamp(cc[0], 0.0, float(HO - 1))
        x_cl[b, k] = _smooth_clamp(cc[1], 0.0, float(WO - 1))

    y_idx = np.rint(y_cl).astype(np.int32)
    x_idx = np.rint(x_cl).astype(np.int32)
    b_idx = np.arange(BS)[:, None]
    k_idx = np.arange(K)[None, :]
    off_y = offset_map[b_idx, 2 * k_idx, y_idx, x_idx]
    off_x = offset_map[b_idx, 2 * k_idx + 1, y_idx, x_idx]
    offset = np.stack([off_y, off_x], axis=-1)

    pts = (coords.astype(np.float32) + offset) * STRIDE
    return pts.astype(np.float32)
